# revision 16
# baseline (speedup 1.0000x reference)
"""Adaptive 5x5 per-pixel convolution on 8 TRN2 NeuronCores.

out[b,c,h,w] = sum_{i,j} x[b,c,h+i,w+j] * kernel[b,i*5+j,h,w]

Sharding: data-parallel over batch B=8 -> 1 batch per core.
Per-core: x [64, 260, 260], kernel [25, 256, 256] -> out [64, 256, 256].

v2 design (vs v1): all row alignment moved into DMA so there is no tail
and no shifted-identity stationaries.

- Two 128-row output blocks (h0 = 0, 128). For tap-row i, the x tile is
  DMA'd with partition p = x row h0+i+p (DMA may start at any partition,
  engines may only read 32-aligned partition bases). k tiles are fully
  aligned: tile i, slot j holds k[(i,j), h0+p, w].
- One DVE tensor_mul per (block, cgroup, i) computes all 5 j-slot
  products [128, (5 j, 8 c, 256 w)]: in0 = x read at slot stride 1 (slot
  j reads x[.., w+j]; odd starts still hit DVE 2x mode - measured),
  in1 = k (stride-0 over c). This is the bottleneck and runs at 97% of
  the DVE 2x roofline (25 products/output * 4.19M / 245.8G/s = 427us).
  GPSIMD offload of one tap row was tried and removed: concurrent
  Pool+DVE tensor_tensor contend for SBUF ports and cut DVE to ~1/4
  speed (combined throughput below DVE alone).
- Product slot j holds the tap-(i,j) contribution to out row h0+p at w.
  TensorE accumulates all 25 taps per channel pair into PSUM with the
  same identity stationary. 4 PSUM accumulators x bufs=2 = 8 banks.
- ScalarE copies PSUM->SBUF bf16; k/stile DMAs trigger on the idle
  gpsimd queue so x DMAs own the sync queue.
- Host: cast to bf16, relayout x -> [h, c, w], kernel -> [h, i, j, w],
  out <- [h, c, w].
"""

import sys
from contextlib import ExitStack

import ml_dtypes
import numpy as np

sys.path.insert(0, "/opt/trn_rl_repo")

from concourse import bacc, bass, tile  # noqa: E402
from concourse import mybir  # noqa: E402
from concourse.bass_utils import run_bass_kernel_spmd  # noqa: E402

F32 = mybir.dt.float32
BF16 = mybir.dt.bfloat16
BF16_NP = ml_dtypes.bfloat16

C, HP, WP = 64, 260, 260
KK, H, W = 25, 256, 256
K = 5

XR = 128  # rows per block
CG = 16  # channels per x/product tile
V = 260  # j-slot width (256 valid + up to 4 shift overhang)

_CACHE = {}


def _ap(t, off, dims):
    return bass.AP(t[:].tensor, off, dims)


def _build_nc():
    nc = bacc.Bacc(
        "TRN2", target_bir_lowering=False, debug=False, num_devices=8
    )
    x = nc.dram_tensor("x", [HP, C, WP], BF16, kind="ExternalInput").ap()
    k = nc.dram_tensor("k", [H, K, K, W], BF16, kind="ExternalInput").ap()
    s = nc.dram_tensor("s", [XR, XR], BF16, kind="ExternalInput").ap()
    out = nc.dram_tensor("out", [H, C, W], BF16, kind="ExternalOutput").ap()

    with tile.TileContext(nc) as tc, ExitStack() as ctx:
        spool = ctx.enter_context(tc.tile_pool(name="spool", bufs=1))
        ktpool = ctx.enter_context(tc.tile_pool(name="ktpool", bufs=1))
        xpool = ctx.enter_context(tc.tile_pool(name="xpool", bufs=2))
        ppool = ctx.enter_context(tc.tile_pool(name="ppool", bufs=2))
        opool = ctx.enter_context(tc.tile_pool(name="opool", bufs=8))
        mmpool = ctx.enter_context(tc.tile_pool(name="mm", bufs=2, space="PSUM"))

        stile = spool.tile([XR, XR], BF16)

        xt0_first = None
        for h0 in (0, XR):
            if h0 == 0:
                # first x tile issued before the k tiles: the first DVE op
                # (the critical path) starts as soon as it lands
                xt0_first = xpool.tile([XR, CG * V], BF16, tag="x0", name="xt0f")
                nc.sync.dma_start(
                    xt0_first[:].rearrange("p (c w) -> p c w", c=CG),
                    x[0:XR, 0:CG, :],
                )
            # k tiles: [128, 5*256]; slot j holds k[(i,j), h0+p, w].
            # Triggered on the idle gpsimd queue so x DMAs own the sync queue.
            ktiles = []
            for i in range(K):
                kt = ktpool.tile([XR, K * W], BF16, tag=f"k{i}")
                nc.gpsimd.dma_start(
                    kt[:].rearrange("p (j w) -> p j w", j=K),
                    k[h0 : h0 + XR, i, :, :],
                )
                ktiles.append(kt)
            if h0 == 0:
                nc.gpsimd.dma_start(stile[:], s[:])

            for c0 in range(0, C, CG):
                # x tiles, one per tap-row i, partition p = x row h0+i+p
                xts_l = []
                for i in range(K):
                    if h0 == 0 and c0 == 0 and i == 0:
                        xts_l.append((0, xt0_first))
                        continue
                    xt = xpool.tile([XR, CG * V], BF16, tag=f"x{i}", name=f"xt{i}")
                    nc.sync.dma_start(
                        xt[:].rearrange("p (c w) -> p c w", c=CG),
                        x[h0 + i : h0 + i + XR, c0 : c0 + CG, :],
                    )
                    xts_l.append((i, xt))

                xts = dict(xts_l)
                pss = [
                    mmpool.tile([XR, 2 * W], F32, tag=f"ps{cp}", name=f"ps{cp}", bufs=1)
                    for cp in range(CG // 2)
                ]

                last_iter = h0 == XR and c0 == C - CG
                for i in range(K):
                    pt = ppool.tile([XR, K * CG * W], BF16, tag="p", name="pt")
                    # split the very last product op so the PE drain overlaps it
                    halves = (
                        [(cb, 4) for cb in range(0, CG, 4)]
                        if (last_iter and i == K - 1)
                        else [(0, CG)]
                    )
                    for cb, cn in halves:
                        nc.vector.tensor_mul(
                            _ap(pt, cb * W, [[K * CG * W, XR], [CG * W, K], [W, cn], [1, W]]),
                            _ap(xts[i], cb * V, [[CG * V, XR], [1, K], [V, cn], [1, W]]),
                            _ap(ktiles[i], 0, [[K * W, XR], [W, K], [0, cn], [1, W]]),
                        )
                        for cp in range(cb // 2, (cb + cn) // 2):
                            for j in range(K):
                                mv = _ap(
                                    pt,
                                    j * CG * W + cp * 2 * W,
                                    [[K * CG * W, XR], [W, 2], [1, W]],
                                )
                                nc.tensor.matmul(
                                    pss[cp][:],
                                    stile[:],
                                    mv,
                                    start=(i == 0 and j == 0),
                                    stop=(i == K - 1 and j == K - 1),
                                )

                for cp in range(CG // 2):
                    ot = opool.tile([XR, 2 * W], BF16)
                    nc.scalar.copy(ot[:], pss[cp][:])
                    nc.sync.dma_start(
                        out[h0 : h0 + XR, c0 + 2 * cp : c0 + 2 * cp + 2, :],
                        ot[:].rearrange("p (c w) -> p c w", c=2),
                    )

    nc.compile()
    return nc


def _get_nc():
    if "nc" not in _CACHE:
        _CACHE["nc"] = _build_nc()
    return _CACHE["nc"]


def run(x, kernel, trace=False):
    """x: [8,64,260,260] f32, kernel: [8,25,256,256] f32 -> ([8,64,256,256], exec_ns)."""
    nc = _get_nc()
    xb = np.asarray(x).astype(BF16_NP)
    kb = np.asarray(kernel).astype(BF16_NP)
    sc = np.eye(XR, dtype=np.float32).astype(BF16_NP)
    in_maps = []
    for b in range(8):
        xr = np.ascontiguousarray(xb[b].transpose(1, 0, 2))  # [h, c, w]
        kr = np.ascontiguousarray(
            kb[b].reshape(K, K, H, W).transpose(2, 0, 1, 3)
        )  # [h, i, j, w]
        in_maps.append({"x": xr, "k": kr, "s": sc})
    res = run_bass_kernel_spmd(nc, in_maps, core_ids=list(range(8)), trace=trace)
    outs = []
    for b in range(8):
        o = np.asarray(res.results[b]["out"], dtype=np.float32)  # [h, c, w]
        outs.append(o.transpose(1, 0, 2))
    return np.ascontiguousarray(np.stack(outs, axis=0)), res.exec_time_ns


def kernel(**inputs):
    out, _ = run(inputs["x"], inputs["kernel"], trace=False)
    return out


# revision 17
# speedup vs baseline: 1.0138x; 1.0138x over previous
"""Adaptive 5x5 per-pixel convolution on 8 TRN2 NeuronCores.

out[b,c,h,w] = sum_{i,j} x[b,c,h+i,w+j] * kernel[b,i*5+j,h,w]

Sharding: data-parallel over batch B=8 -> 1 batch per core.
Per-core: x [64, 260, 260], kernel [25, 256, 256] -> out [64, 256, 256].

v2 design (vs v1): all row alignment moved into DMA so there is no tail
and no shifted-identity stationaries.

- Two 128-row output blocks (h0 = 0, 128). For tap-row i, the x tile is
  DMA'd with partition p = x row h0+i+p (DMA may start at any partition,
  engines may only read 32-aligned partition bases). k tiles are fully
  aligned: tile i, slot j holds k[(i,j), h0+p, w].
- One DVE tensor_mul per (block, cgroup, i) computes all 5 j-slot
  products [128, (5 j, 8 c, 256 w)]: in0 = x read at slot stride 1 (slot
  j reads x[.., w+j]; odd starts still hit DVE 2x mode - measured),
  in1 = k (stride-0 over c). This is the bottleneck and runs at 97% of
  the DVE 2x roofline (25 products/output * 4.19M / 245.8G/s = 427us).
  GPSIMD offload of one tap row was tried and removed: concurrent
  Pool+DVE tensor_tensor contend for SBUF ports and cut DVE to ~1/4
  speed (combined throughput below DVE alone).
- Product slot j holds the tap-(i,j) contribution to out row h0+p at w.
  TensorE accumulates all 25 taps per channel pair into PSUM with the
  same identity stationary. 4 PSUM accumulators x bufs=2 = 8 banks.
- ScalarE copies PSUM->SBUF bf16; k/stile DMAs trigger on the idle
  gpsimd queue so x DMAs own the sync queue.
- Host: cast to bf16, relayout x -> [h, c, w], kernel -> [h, i, j, w],
  out <- [h, c, w].
"""

import sys
from contextlib import ExitStack

import ml_dtypes
import numpy as np

sys.path.insert(0, "/opt/trn_rl_repo")

from concourse import bacc, bass, tile  # noqa: E402
from concourse import mybir  # noqa: E402
from concourse.bass_utils import run_bass_kernel_spmd  # noqa: E402

F32 = mybir.dt.float32
BF16 = mybir.dt.bfloat16
BF16_NP = ml_dtypes.bfloat16

C, HP, WP = 64, 260, 260
KK, H, W = 25, 256, 256
K = 5

XR = 128  # rows per block
CG = 8  # channels per x/product tile
V = 260  # j-slot width (256 valid + up to 4 shift overhang)

_CACHE = {}


def _ap(t, off, dims):
    return bass.AP(t[:].tensor, off, dims)


def _build_nc():
    nc = bacc.Bacc(
        "TRN2", target_bir_lowering=False, debug=False, num_devices=8
    )
    x = nc.dram_tensor("x", [HP, C, WP], BF16, kind="ExternalInput").ap()
    k = nc.dram_tensor("k", [H, K, K, W], BF16, kind="ExternalInput").ap()
    s = nc.dram_tensor("s", [XR, XR], BF16, kind="ExternalInput").ap()
    out = nc.dram_tensor("out", [H, C, W], BF16, kind="ExternalOutput").ap()

    with tile.TileContext(nc) as tc, ExitStack() as ctx:
        spool = ctx.enter_context(tc.tile_pool(name="spool", bufs=1))
        ktpool = ctx.enter_context(tc.tile_pool(name="ktpool", bufs=2))
        xpool = ctx.enter_context(tc.tile_pool(name="xpool", bufs=2))
        ppool = ctx.enter_context(tc.tile_pool(name="ppool", bufs=4))
        opool = ctx.enter_context(tc.tile_pool(name="opool", bufs=8))
        mmpool = ctx.enter_context(tc.tile_pool(name="mm", bufs=2, space="PSUM"))

        stile = spool.tile([XR, XR], BF16)

        xt0_first = None
        for h0 in (0, XR):
            if h0 == 0:
                # first x tile issued before the k tiles: the first DVE op
                # (the critical path) starts as soon as it lands
                xt0_first = xpool.tile([XR, CG * V], BF16, tag="x0", name="xt0f")
                nc.sync.dma_start(
                    xt0_first[:].rearrange("p (c w) -> p c w", c=CG),
                    x[0:XR, 0:CG, :],
                )
            # k tiles: [128, 5*256]; slot j holds k[(i,j), h0+p, w].
            # Triggered on the idle gpsimd queue so x DMAs own the sync queue.
            ktiles = []
            for i in range(K):
                kt = ktpool.tile([XR, K * W], BF16, tag=f"k{i}")
                nc.gpsimd.dma_start(
                    kt[:].rearrange("p (j w) -> p j w", j=K),
                    k[h0 : h0 + XR, i, :, :],
                )
                ktiles.append(kt)
            if h0 == 0:
                nc.gpsimd.dma_start(stile[:], s[:])

            for c0 in range(0, C, CG):
                # x tiles, one per tap-row i, partition p = x row h0+i+p
                xts_l = []
                for i in range(K):
                    if h0 == 0 and c0 == 0 and i == 0:
                        xts_l.append((0, xt0_first))
                        continue
                    xt = xpool.tile([XR, CG * V], BF16, tag=f"x{i}", name=f"xt{i}")
                    nc.sync.dma_start(
                        xt[:].rearrange("p (c w) -> p c w", c=CG),
                        x[h0 + i : h0 + i + XR, c0 : c0 + CG, :],
                    )
                    xts_l.append((i, xt))

                xts = dict(xts_l)
                pss = [
                    mmpool.tile([XR, 2 * W], F32, tag=f"ps{cp}", name=f"ps{cp}")
                    for cp in range(CG // 2)
                ]

                last_iter = h0 == XR and c0 == C - CG
                for i in range(K):
                    pt = ppool.tile([XR, K * CG * W], BF16, tag="p", name="pt")
                    # split the very last product op so the PE drain overlaps it
                    halves = (
                        [(cb, 2) for cb in range(0, CG, 2)]
                        if (last_iter and i == K - 1)
                        else [(0, CG)]
                    )
                    for cb, cn in halves:
                        nc.vector.tensor_mul(
                            _ap(pt, cb * W, [[K * CG * W, XR], [CG * W, K], [W, cn], [1, W]]),
                            _ap(xts[i], cb * V, [[CG * V, XR], [1, K], [V, cn], [1, W]]),
                            _ap(ktiles[i], 0, [[K * W, XR], [W, K], [0, cn], [1, W]]),
                        )
                        for cp in range(cb // 2, (cb + cn) // 2):
                            for j in range(K):
                                mv = _ap(
                                    pt,
                                    j * CG * W + cp * 2 * W,
                                    [[K * CG * W, XR], [W, 2], [1, W]],
                                )
                                nc.tensor.matmul(
                                    pss[cp][:],
                                    stile[:],
                                    mv,
                                    start=(i == 0 and j == 0),
                                    stop=(i == K - 1 and j == K - 1),
                                )

                for cp in range(CG // 2):
                    ot = opool.tile([XR, 2 * W], BF16)
                    nc.scalar.copy(ot[:], pss[cp][:])
                    nc.sync.dma_start(
                        out[h0 : h0 + XR, c0 + 2 * cp : c0 + 2 * cp + 2, :],
                        ot[:].rearrange("p (c w) -> p c w", c=2),
                    )

    nc.compile()
    return nc


def _get_nc():
    if "nc" not in _CACHE:
        _CACHE["nc"] = _build_nc()
    return _CACHE["nc"]


def run(x, kernel, trace=False):
    """x: [8,64,260,260] f32, kernel: [8,25,256,256] f32 -> ([8,64,256,256], exec_ns)."""
    nc = _get_nc()
    xb = np.asarray(x).astype(BF16_NP)
    kb = np.asarray(kernel).astype(BF16_NP)
    sc = np.eye(XR, dtype=np.float32).astype(BF16_NP)
    in_maps = []
    for b in range(8):
        xr = np.ascontiguousarray(xb[b].transpose(1, 0, 2))  # [h, c, w]
        kr = np.ascontiguousarray(
            kb[b].reshape(K, K, H, W).transpose(2, 0, 1, 3)
        )  # [h, i, j, w]
        in_maps.append({"x": xr, "k": kr, "s": sc})
    res = run_bass_kernel_spmd(nc, in_maps, core_ids=list(range(8)), trace=trace)
    outs = []
    for b in range(8):
        o = np.asarray(res.results[b]["out"], dtype=np.float32)  # [h, c, w]
        outs.append(o.transpose(1, 0, 2))
    return np.ascontiguousarray(np.stack(outs, axis=0)), res.exec_time_ns


def kernel(**inputs):
    out, _ = run(inputs["x"], inputs["kernel"], trace=False)
    return out
